# revision 1
# baseline (speedup 1.0000x reference)
"""Trainium2 Bass kernel for nn_BoxEncoder (B=128, T=200, NC=3, NB=2, D=512, DH=256).

Data-parallel over batch: 16 batch items per core x 8 cores. Token layout per
core: partition p = bt*8 + q (bt = batch item 0..15, q = 0..7). Each partition
owns 225 j-slots: j in [0,75) dist tokens, j in [75,225) box tokens.

Pipeline (v2, ~2x faster than v1):
 - LayerNorm mean folded into W1 on the host (W1c = W1 - rowmean), so
   z = x @ W1c is already centered and the LN needs no bias.
 - LN variance via the Gram trick: var = x^T G x / 256 with G = W1c W1c^T
   precomputed on host as a 4x block-diagonal [128,128], one K=128 matmul
   per 4 tiles plus a DVE mult + segmented reduce -- z is computed exactly
   once (v1 ran the whole z matmul twice).
 - rstd batched in two halves: one ACT Sqrt + one DVE reciprocal each
   (exactly two ACT table loads in the whole kernel: Sqrt set, Gelu set).
 - gelu(z * rstd) applied straight from PSUM with the per-partition scale
   AP; h written to SBUF bf16 in groups of 3 tiles.
 - h transposed with dma_start_transpose (SBUF->SBUF, 3 tiles per issue)
   instead of PE transposes + PSUM round-trip copies. The W2 consumers run
   4 groups behind the transpose issue to hide the DMA+sem latency.
 - W2 accumulation (hi/lo/extras) as a rolling skew: slot s issues
   [hi_s, lo_{s-1}, x_{s-2}, copy_{s-2}] so same-bank accumulating matmuls
   are never adjacent (PSUM accumulate drain would stall them).
 - dist tiles (rank-1 tokens) run in the Gram phase as PE fillers.
 - Output staged and DMA'd as bf16 (halves HBM write traffic); host
   converts back to f32 (~0.2% extra rel err, budget is 2e-2). Staged out
   DMAs are deferred one iteration so their SP-queue waits are always
   satisfied and never head-of-line-block the transpose issues.

Hardware pitfalls baked into the structure (found the hard way):
 - matmul cannot target a 3D PSUM tile slice, and a nonzero tile_position
   requires a bank-aligned PSUM output.
 - tensor_tensor_reduce and two-PSUM-operand tensor_tensor fail NEFF
   compile/exec; use mult + tensor_reduce and single-PSUM operands.
 - accumulation chains run ~311ns/matmul vs 216ns for singles/pairs at
   N=512; z/dist/extras singles issue at full rate.
"""

import numpy as np
import ml_dtypes

B, T, NCAM, NB, D, DH = 128, 200, 3, 2, 512, 256
IW, IH = 640.0, 400.0
NCORES = 8
BPC = B // NCORES            # batch items per core
JD, JB = 75, 150             # dist / box j-slots per partition
J = JD + JB                  # 225
F = 32                       # feature columns per j-slot
NCHUNK = (J * F + 127) // 128   # 57 transpose chunks (56 full + 1 of 32 cols)
NG = JB // 3                 # 50 box groups of 3 tiles

_CACHE = {}


def _build_nc():
    from contextlib import ExitStack
    import concourse.bacc as bacc
    import concourse.mybir as mybir
    import concourse.tile as tile

    f32 = mybir.dt.float32
    bf16 = mybir.dt.bfloat16
    A = mybir.AluOpType
    AF = mybir.ActivationFunctionType

    # bf16 pack column offsets
    C_W1 = 0
    C_W2HI = C_W1 + 256
    C_W2LO = C_W2HI + 512
    C_W2X = C_W2LO + 512          # 3 cam variants, 512 each
    C_G = C_W2X + 3 * 512
    C_ID = C_G + 128
    NBF = C_ID + 128

    nc = bacc.Bacc("TRN2", target_bir_lowering=False, debug=False,
                   num_devices=NCORES)
    stg_dt = bf16
    fpk = nc.declare_dram_parameter("fpk", [128, 900 + 128], f32, isOutput=False)
    bpk = nc.declare_dram_parameter("bpk", [128, NBF], bf16, isOutput=False)
    out_d = nc.declare_dram_parameter("out", [BPC, 1800, D], bf16, isOutput=True)

    with ExitStack() as ctx:
        tc = ctx.enter_context(tile.TileContext(nc))
        cp = ctx.enter_context(tc.tile_pool(name="const", bufs=1))
        sc = ctx.enter_context(tc.tile_pool(name="scratch", bufs=1))
        # PSUM pools (8 banks):  zg 2x2 + op 3x1 + ctp/yp slack
        zgp = ctx.enter_context(tc.tile_pool(name="zgp", bufs=3, space="PSUM"))
        opa = ctx.enter_context(tc.tile_pool(name="opa", bufs=5, space="PSUM"))
        tmpp = ctx.enter_context(tc.tile_pool(name="tmpp", bufs=2))
        hgp = ctx.enter_context(tc.tile_pool(name="hgp", bufs=3))
        htp = ctx.enter_context(tc.tile_pool(name="htp", bufs=5))
        wstp = ctx.enter_context(tc.tile_pool(name="wstp", bufs=2))
        bstg = ctx.enter_context(tc.tile_pool(name="bstage", bufs=3))
        dstg = ctx.enter_context(tc.tile_pool(name="dstage", bufs=3))

        fpack = cp.tile([128, 900 + 128], f32)
        nc.sync.dma_start(fpack[:], fpk[:])
        bpack = cp.tile([128, NBF], bf16)
        nc.sync.dma_start(bpack[:], bpk[:])

        raw = fpack[:, 0:900]
        idf = fpack[:, 900:1028]
        w1 = bpack[:, C_W1:C_W1 + 256]
        w2hi = bpack[:, C_W2HI:C_W2HI + 512]
        w2lo = bpack[:, C_W2LO:C_W2LO + 512]
        w2x = [bpack[:, C_W2X + c * 512: C_W2X + (c + 1) * 512] for c in range(3)]
        Gblk = bpack[:, C_G:C_G + 128]

        TF = cp.tile([128, J * F], f32)
        nc.gpsimd.memset(TF[:], 0.0)

        TFj = TF.rearrange("p (j f) -> p j f", f=F)
        TFd = TFj[:, :JD, :]                       # dist slots
        TFb = TFj[:, JD:, :]                       # box slots
        TFbp = TF[:, JD * F:].rearrange("p (m g f) -> p m g f", g=2, f=F)
        raw6 = raw.rearrange("p (b s) -> p b s", s=6)
        rawp = raw.rearrange("p (m g s) -> p m g s", g=2, s=6)

        # ---------------- P1: feature planes ----------------
        sPres = sc.tile([128, JB], f32)
        sKey = sc.tile([128, JB], f32)
        sSwap = sc.tile([128, JD], f32)
        sD = sc.tile([128, JD], f32)
        sSD = sc.tile([128, JD], f32)
        sw = [sc.tile([128, JB], f32, tag=f"swp{i}", name=f"swp{i}")
              for i in range(6)]
        sT0 = sc.tile([128, JB], f32)
        sT1 = sc.tile([128, JB], f32)

        nc.vector.tensor_tensor(sT0[:], raw6[:, :, 0], raw6[:, :, 1], A.add)
        nc.vector.tensor_tensor(sT1[:], raw6[:, :, 2], raw6[:, :, 3], A.add)
        nc.vector.tensor_tensor(sT0[:], sT0[:], sT1[:], A.add)
        nc.vector.tensor_scalar(sPres[:], sT0[:], 0.0, None, A.not_equal)
        # key = cat - 1000*pres  (order-equivalent to cat + 1000*(1-pres))
        nc.vector.scalar_tensor_tensor(sKey[:], sPres[:], -1000.0,
                                       raw6[:, :, 4], A.mult, A.add)
        sKeyp = sKey.rearrange("p (m g) -> p m g", g=2)
        nc.vector.tensor_tensor(sSwap[:], sKeyp[:, :, 1], sKeyp[:, :, 0], A.is_lt)

        # compare-and-swap each of the 6 raw components + presence
        for i in range(6):
            ve, vo = rawp[:, :, 0, i], rawp[:, :, 1, i]
            dst = sw[i].rearrange("p (m g) -> p m g", g=2)
            nc.vector.tensor_tensor(sD[:], vo, ve, A.subtract)
            nc.vector.tensor_tensor(sSD[:], sD[:], sSwap[:], A.mult)
            nc.vector.tensor_tensor(dst[:, :, 0], ve, sSD[:], A.add)
            nc.vector.tensor_tensor(dst[:, :, 1], vo, sSD[:], A.subtract)
        sPresP = sPres.rearrange("p (m g) -> p m g", g=2)
        nc.vector.tensor_tensor(sD[:], sPresP[:, :, 1], sPresP[:, :, 0], A.subtract)
        nc.vector.tensor_tensor(sSD[:], sD[:], sSwap[:], A.mult)
        nc.vector.tensor_tensor(TFbp[:, :, 0, 14], sPresP[:, :, 0], sSD[:], A.add)
        nc.vector.tensor_tensor(TFbp[:, :, 1, 14], sPresP[:, :, 1], sSD[:], A.subtract)

        sX1, sY1, sX2, sY2, sCat, sConf = sw
        # f0..f3: normalized coords
        nc.vector.tensor_scalar(TFb[:, :, 0], sX1[:], 1.0 / IW, None, A.mult)
        nc.vector.tensor_scalar(TFb[:, :, 1], sY1[:], 1.0 / IH, None, A.mult)
        nc.vector.tensor_scalar(TFb[:, :, 2], sX2[:], 1.0 / IW, None, A.mult)
        nc.vector.tensor_scalar(TFb[:, :, 3], sY2[:], 1.0 / IH, None, A.mult)
        # f4 w, f5 h, f6 cx*2, f7 cy*2 (the 0.5 is folded into the weights)
        nc.vector.tensor_tensor(TFb[:, :, 4], TFb[:, :, 2], TFb[:, :, 0], A.subtract)
        nc.vector.tensor_tensor(TFb[:, :, 5], TFb[:, :, 3], TFb[:, :, 1], A.subtract)
        nc.vector.tensor_tensor(TFb[:, :, 6], TFb[:, :, 0], TFb[:, :, 2], A.add)
        nc.vector.tensor_tensor(TFb[:, :, 7], TFb[:, :, 1], TFb[:, :, 3], A.add)
        # f8 area, f9 aspect = w / (h + 1e-6)
        nc.vector.tensor_tensor(TFb[:, :, 8], TFb[:, :, 4], TFb[:, :, 5], A.mult)
        sHp = sT0
        nc.vector.tensor_scalar(sHp[:], TFb[:, :, 5], 1e-6, None, A.add)
        sR = sT1
        nc.vector.reciprocal(sR[:], sHp[:])
        nc.vector.tensor_tensor(TFb[:, :, 9], TFb[:, :, 4], sR[:], A.mult)
        # f10..12 cat one-hots * pres ; f13 conf*pres ; f15 = 1-pres
        for k in range(3):
            nc.vector.scalar_tensor_tensor(TFb[:, :, 10 + k], sCat[:], float(k),
                                           TFb[:, :, 14], A.is_equal, A.mult)
        nc.vector.tensor_tensor(TFb[:, :, 13], sConf[:], TFb[:, :, 14], A.mult)
        nc.vector.tensor_scalar(TFb[:, :, 15], TFb[:, :, 14], -1.0, 1.0,
                                A.mult, A.add)
        # dist tokens: f16 = 0.5*sqrt(dx2^2+dy2^2) (cx stored doubled), f17 = 1
        sDx = sc.tile([128, JD], f32)
        sDy = sc.tile([128, JD], f32)
        nc.vector.tensor_tensor(sDx[:], TFbp[:, :, 0, 6], TFbp[:, :, 1, 6], A.subtract)
        nc.vector.tensor_tensor(sDy[:], TFbp[:, :, 0, 7], TFbp[:, :, 1, 7], A.subtract)
        nc.vector.tensor_tensor(sDx[:], sDx[:], sDx[:], A.mult)
        nc.vector.tensor_tensor(sDy[:], sDy[:], sDy[:], A.mult)
        nc.vector.tensor_tensor(sDx[:], sDx[:], sDy[:], A.add)
        nc.scalar.activation(TFd[:, :, 16], sDx[:], AF.Sqrt, scale=0.25)
        nc.vector.memset(TFd[:, :, 17], 1.0)

        # ---------------- P2: transpose T_feat chunks -> bf16 lhsT tiles ----
        cta = cp.tile([128, NCHUNK * 128], bf16)
        # garbage rows of the last (short) chunk hit zero blocks of Gblk, but
        # must at least be finite: zero them once
        nc.vector.memset(cta[:, (NCHUNK - 1) * 128:], 0.0)
        for ci in range(NCHUNK):
            w_cols = min(128, J * F - ci * 128)
            ps = opa.tile([128, D], f32, tag="oa", name="oa")[:, 0:128]
            nc.tensor.transpose(ps[:w_cols, :], TF[:, ci * 128: ci * 128 + w_cols],
                                idf)
            dst = cta[:w_cols, ci * 128: ci * 128 + 128]
            if ci % 2 == 0:
                nc.vector.tensor_copy(dst, ps[:w_cols, :])
            else:
                nc.scalar.copy(dst, ps[:w_cols, :])

        def lhsT(j):
            ci, jj = j // 4, j % 4
            return cta[32 * jj: 32 * jj + 32, ci * 128: (ci + 1) * 128]

        # ---------------- P3: Gram variance + dist tiles ----------------
        v = sc.tile([128, JB], f32)

        dist_copy_idx = 0
        dist_stage = None
        vd = out_d[:, 0:600, :].rearrange("b (q r) d -> b q r d", q=8)

        eps = sc.tile([128, 1], f32)
        nc.vector.memset(eps[:], 1e-5)
        sd = sc.tile([128, JB], f32)
        rstd = sc.tile([128, JB], f32)

        def emit_rstd(k0, k1):
            nc.scalar.activation(sd[:, k0:k1], v[:, k0:k1], AF.Sqrt,
                                 bias=eps[:], scale=1.0 / DH)
            nc.vector.reciprocal(rstd[:, k0:k1], sd[:, k0:k1])

        def emit_dist_tile(jd):
            jjd = jd % 4
            o = opa.tile([128, D], f32, tag="oa", name="oa")
            nc.tensor.matmul(o[:], lhsT(jd), w2x[0][32 * jjd: 32 * jjd + 32, :],
                             start=True, stop=True,
                             tile_position=(32 * jjd, 0))
            return o

        for ci in range(JD // 4, NCHUNK):
            y4t = opa.tile([128, D], f32, tag="oa", name="oa")
            y4 = y4t[:, 0:128]
            nc.tensor.matmul(y4, cta[:, ci * 128:(ci + 1) * 128], Gblk,
                             start=True, stop=True)
            j0, j1 = max(4 * ci, JD), min(4 * ci + 4, J)
            if j1 - j0 == 4:
                tmp = tmpp.tile([128, 128], f32, tag="tmp", name="tmp")
                nc.vector.tensor_tensor(tmp[:], TF[:, 4 * ci * F:(4 * ci + 4) * F],
                                        y4[:], A.mult)
                nc.vector.tensor_reduce(v[:, j0 - JD:j1 - JD],
                                        tmp.rearrange("p (j f) -> p j f", f=F),
                                        mybir.AxisListType.X, A.add)
            else:
                for j in range(j0, j1):
                    k = j - JD
                    waste = wstp.tile([128, 32], f32, tag="wst", name="waste")
                    nc.vector.tensor_tensor(waste[:], TFj[:, j, :],
                                            y4[:, 32 * (j % 4):32 * (j % 4) + 32],
                                            A.mult)
                    nc.vector.tensor_reduce(v[:, k:k + 1], waste[:],
                                            mybir.AxisListType.X, A.add)
            # interleave 2 dist tiles per chunk
            for _ in range(2):
                if dist_copy_idx >= JD:
                    continue
                jd = dist_copy_idx
                o = emit_dist_tile(jd)
                if dist_stage is None:
                    dist_stage = dstg.tile([128, 5 * D], stg_dt, tag="dstage",
                                           name="dist_stage")
                slot = jd % 5
                if jd % 3 == 2:
                    nc.vector.tensor_copy(dist_stage[:, slot * D:(slot + 1) * D], o[:])
                else:
                    nc.scalar.copy(dist_stage[:, slot * D:(slot + 1) * D], o[:])
                dist_copy_idx += 1
                if slot == 4:
                    nc.sync.dma_start(vd[:, :, jd - 4:jd + 1, :], dist_stage[:])
                    dist_stage = None
            if ci == 37:
                emit_rstd(0, 75)
        # leftover dist tiles (39 chunks x 2 = 78 >= 75, none left normally)
        while dist_copy_idx < JD:
            jd = dist_copy_idx
            o = emit_dist_tile(jd)
            if dist_stage is None:
                dist_stage = dstg.tile([128, 5 * D], stg_dt, tag="dstage",
                                       name="dist_stage")
            slot = jd % 5
            nc.scalar.copy(dist_stage[:, slot * D:(slot + 1) * D], o[:])
            dist_copy_idx += 1
            if slot == 4:
                nc.sync.dma_start(vd[:, :, jd - 4:jd + 1, :], dist_stage[:])
                dist_stage = None

        # ---------------- P3b: second rstd batch ----------------
        emit_rstd(75, JB)

        # ---------------- P4: box pipeline (groups of 3 tiles) ----------------
        # Per tile: accumulation PAIR (w2hi+w2lo) into bank A at full rate,
        # extras as an independent SINGLE into bank B, combined by the DVE
        # staging copy (A+B -> bf16). 3-chains run at ~311ns/matmul on hw,
        # pairs and singles at ~216ns.
        # Iteration order keeps the in-order PE queue stall-free:
        #   [gelu g-1 (ACT), transpose g-1 (SP)] [W slots g-2 (PE)] [z g (PE)]
        zg_t, ht_t, oa_t, ob_t = {}, {}, {}, {}
        stage_state = {"tile": None, "fill": 0, "pending": None}
        vb = out_d[:, 600:1800, :].rearrange("b (q r) d -> b q r d", q=8)

        def emit_slot(s_):
            if s_ < JB:                      # hi_s (start)
                oa_t[s_] = opa.tile([128, D], f32, tag="oa", name="oa")
                ht = ht_t[s_ // 3]
                nc.tensor.matmul(oa_t[s_][:], ht[:, 2 * (s_ % 3), :], w2hi,
                                 start=True, stop=False)
            k = s_ - 1
            if 0 <= k < JB:                  # lo_{s-1}
                ht = ht_t[k // 3]
                nc.tensor.matmul(oa_t[k][:], ht[:, 2 * (k % 3) + 1, :], w2lo,
                                 start=False, stop=False)
                if k % 3 == 2:
                    ht_t.pop(k // 3)
            k = s_ - 2
            if 0 <= k < JB:                  # x_{s-2} (stop) + copy
                j = JD + k
                jj = j % 4
                cam = (k % 6) // 2
                ot = oa_t.pop(k)
                nc.tensor.matmul(ot[:], lhsT(j),
                                 w2x[cam][32 * jj:32 * jj + 32, :],
                                 start=False, stop=True,
                                 tile_position=(32 * jj, 0))
                if stage_state["tile"] is None:
                    stage_state["tile"] = bstg.tile([128, 8 * D], stg_dt,
                                                    tag="bstage", name="bstage")
                    stage_state["fill"] = 0
                fill = stage_state["fill"]
                dst = stage_state["tile"][:, fill * D:(fill + 1) * D]
                if k % 2 == 0:
                    nc.vector.tensor_copy(dst, ot[:])
                else:
                    nc.scalar.copy(dst, ot[:])
                stage_state["fill"] = fill + 1
                if stage_state["fill"] == 8 or k == JB - 1:
                    gsz = stage_state["fill"]
                    j0 = k - gsz + 1
                    stage_state["pending"] = (stage_state["tile"], j0, gsz)
                    stage_state["tile"] = None

        for it in range(NG + 4):
            # stage G: gelu for group g1 + transpose issue
            g1 = it - 1
            if 0 <= g1 < NG:
                hg = hgp.tile([128, 3, DH], bf16, tag="hg", name="hg")
                for q in range(3):
                    k = 3 * g1 + q
                    zq = zg_t.pop((g1, q))
                    nc.scalar.activation(hg[:, q, :], zq, AF.Gelu,
                                         scale=rstd[:, k:k + 1])
                ht = htp.tile([128, 6, 128], bf16, tag="ht", name="ht")
                ht_t[g1] = ht
                nc.sync.dma_start_transpose(ht[:], hg[:])
            # flush last iteration's staged DMA now that its copies are done
            if stage_state["pending"] is not None:
                ptile, pj0, pgsz = stage_state["pending"]
                nc.sync.dma_start(vb[:, :, pj0:pj0 + pgsz, :], ptile[:, : pgsz * D])
                stage_state["pending"] = None
            # stage W: rolling slots for group g2
            g2 = it - 4
            if 0 <= g2 < NG:
                for q in range(3):
                    emit_slot(3 * g2 + q)
            # stage Z: z matmuls for group g0
            g0 = it
            if g0 < NG:
                for q in range(3):
                    zb = zgp.tile([128, DH], f32, tag="z", name="z")
                    zg_t[(g0, q)] = zb[:]
                    k = 3 * g0 + q
                    j = JD + k
                    jj = j % 4
                    nc.tensor.matmul(zb[:], lhsT(j),
                                     w1[32 * jj:32 * jj + 32, :],
                                     start=True, stop=True,
                                     tile_position=(32 * jj, 0))
        emit_slot(JB)
        emit_slot(JB + 1)
        if stage_state["pending"] is not None:
            ptile, pj0, pgsz = stage_state["pending"]
            nc.sync.dma_start(vb[:, :, pj0:pj0 + pgsz, :], ptile[:, : pgsz * D])
            stage_state["pending"] = None

    nc.compile()
    return nc


def _prep_inputs(inputs):
    f32 = np.float32
    bf = ml_dtypes.bfloat16
    scale = float(np.asarray(inputs["scale"]))

    W1p = np.zeros((32, DH), f32)
    W1p[0:10] = np.asarray(inputs["geom_w1"], f32)
    W1p[6] *= 0.5
    W1p[7] *= 0.5
    W1p -= W1p.mean(axis=1, keepdims=True)      # fold LN mean into W1
    w1rep = np.tile(W1p, (4, 1))

    G = (W1p @ W1p.T).astype(f32)               # gram for LN variance
    Gblk = np.zeros((128, 128), f32)            # block-diag(G x4)
    for t in range(4):
        Gblk[32 * t:32 * t + 32, 32 * t:32 * t + 32] = G

    W2s = scale * np.asarray(inputs["geom_w2"], f32)
    w2hi, w2lo = W2s[:128], W2s[128:]

    cat_t = np.asarray(inputs["cat_table"], f32)
    cam_t = np.asarray(inputs["cam_table"], f32)
    bias_row = (np.asarray(inputs["geom_b2"], f32)
                + np.asarray(inputs["conf_b"], f32)
                + np.asarray(inputs["center_b"], f32))
    w2x_reps = []
    for c in range(3):
        W2x = np.zeros((32, D), f32)
        W2x[6] = scale * np.asarray(inputs["center_w"], f32)[0] * 0.5
        W2x[7] = scale * np.asarray(inputs["center_w"], f32)[1] * 0.5
        W2x[10:13] = scale * cat_t
        W2x[13] = scale * np.asarray(inputs["conf_w"], f32)[0]
        W2x[14] = scale * (bias_row + cam_t[c])
        W2x[15] = np.asarray(inputs["missing_emb"], f32)[0]
        W2x[16] = np.asarray(inputs["dist_w"], f32)[0]
        W2x[17] = np.asarray(inputs["dist_b"], f32)
        w2x_reps.append(np.tile(W2x, (4, 1)))

    idf32 = np.eye(128, dtype=f32)
    bpk = np.concatenate(
        [w1rep, w2hi, w2lo] + w2x_reps + [Gblk, idf32], axis=1
    ).astype(bf)

    box = np.asarray(inputs["box_data"], f32)
    fpks = []
    for c in range(NCORES):
        rawc = box[c * BPC:(c + 1) * BPC].reshape(BPC, T * 6, 6)
        rawc = rawc.reshape(BPC, 8, JB, 6).reshape(128, 900)
        fpks.append(np.ascontiguousarray(
            np.concatenate([rawc, idf32], axis=1), dtype=f32))
    return fpks, bpk


def _fast_path_ok(inputs):
    try:
        shapes = {
            "box_data": (B, T, 6, 6), "cat_table": (3, D), "geom_w1": (10, DH),
            "geom_b1": (DH,), "ln_g": (DH,), "ln_b": (DH,), "geom_w2": (DH, D),
            "geom_b2": (D,), "conf_w": (1, D), "conf_b": (D,),
            "center_w": (2, D), "center_b": (D,), "missing_emb": (1, D),
            "dist_w": (1, D), "dist_b": (D,), "cam_table": (NCAM, D),
        }
        for k, s in shapes.items():
            if tuple(np.asarray(inputs[k]).shape) != s:
                return False
        if not np.all(np.asarray(inputs["geom_b1"]) == 0):
            return False
        if not np.all(np.asarray(inputs["ln_g"]) == 1):
            return False
        if not np.all(np.asarray(inputs["ln_b"]) == 0):
            return False
        return True
    except Exception:
        return False


def _numpy_fallback(inputs):
    # Exact (slow) port of the reference for unexpected inputs.
    import math
    f32 = np.float32
    inp = {k: np.asarray(v) for k, v in inputs.items()}
    coords = inp["box_data"][..., :4].astype(f32)
    category = inp["box_data"][..., 4].astype(np.int32)
    conf = inp["box_data"][..., 5].astype(f32)
    norm = np.array([IW, IH, IW, IH], f32)
    cn = (coords / norm).reshape(B, T, NCAM, NB, 4)
    category = category.reshape(B, T, NCAM, NB)
    conf = conf.reshape(B, T, NCAM, NB, 1)
    presence = (cn.sum(-1) != 0).astype(f32)
    sort_key = category.astype(f32) + (1.0 - presence) * 1000.0
    idx = np.argsort(sort_key, axis=-1, kind="stable")
    cn = np.take_along_axis(cn, idx[..., None], axis=-2)
    category = np.take_along_axis(category, idx, axis=-1)
    conf = np.take_along_axis(conf, idx[..., None], axis=-2)
    presence = (cn.sum(-1) != 0).astype(f32)[..., None]
    x1, y1, x2, y2 = cn[..., 0], cn[..., 1], cn[..., 2], cn[..., 3]
    w, h = x2 - x1, y2 - y1
    cx, cy = (x1 + x2) * 0.5, (y1 + y2) * 0.5
    area, aspect = w * h, w / (h + 1e-6)
    dx, dy = cx[..., 0] - cx[..., 1], cy[..., 0] - cy[..., 1]
    dist = np.sqrt(dx * dx + dy * dy)[..., None]
    dist_tok = dist @ inp["dist_w"].astype(f32) + inp["dist_b"].astype(f32)
    geom = np.stack([x1, y1, x2, y2, w, h, cx, cy, area, aspect], axis=-1)
    z = geom @ inp["geom_w1"].astype(f32) + inp["geom_b1"].astype(f32)
    mu = z.mean(-1, keepdims=True)
    var = ((z - mu) ** 2).mean(-1, keepdims=True)
    xh = (z - mu) / np.sqrt(var + 1e-5) * inp["ln_g"].astype(f32) + inp["ln_b"].astype(f32)
    try:
        from scipy.special import erf as _erf
        g = xh * 0.5 * (1.0 + _erf(xh / np.sqrt(2.0)))
    except Exception:
        verf = np.vectorize(math.erf)
        g = xh * 0.5 * (1.0 + verf(xh / np.sqrt(2.0)))
    geom_p = g @ inp["geom_w2"].astype(f32) + inp["geom_b2"].astype(f32)
    cat_emb = inp["cat_table"].astype(f32)[category]
    conf_p = conf @ inp["conf_w"].astype(f32) + inp["conf_b"].astype(f32)
    center_p = np.stack([cx, cy], axis=-1) @ inp["center_w"].astype(f32) + inp["center_b"].astype(f32)
    cam_emb = inp["cam_table"].astype(f32).reshape(1, 1, NCAM, 1, D)
    tok = (geom_p + cat_emb + conf_p + center_p + cam_emb) * float(inp["scale"])
    tok = np.where(presence == 0, inp["missing_emb"].astype(f32)[0], tok)
    out = np.concatenate([dist_tok.reshape(B, T * NCAM, D),
                          tok.reshape(B, T * NCAM * NB, D)], axis=1)
    return out.astype(np.float32)


def _run(inputs, trace=False, tmpdir=None):
    from concourse.bass_utils import run_bass_kernel_spmd

    if "nc" not in _CACHE:
        _CACHE["nc"] = _build_nc()
    nc = _CACHE["nc"]

    fpks, bpk = _prep_inputs(inputs)
    in_maps = [{"fpk": fpks[c], "bpk": bpk} for c in range(NCORES)]
    res = run_bass_kernel_spmd(nc, in_maps, list(range(NCORES)),
                               trace=trace, tmpdir=tmpdir)
    out = np.concatenate([np.asarray(res.results[c]["out"])
                          for c in range(NCORES)], axis=0)
    return out.astype(np.float32), res


def kernel(**inputs):
    if not _fast_path_ok(inputs):
        return _numpy_fallback(inputs)
    out, _ = _run(inputs)
    return out


if __name__ == "__main__":
    import reference as ref
    inputs = {k: np.asarray(v) for k, v in ref.setup_inputs().items()}
    got = kernel(**inputs)
    exp = np.load("/tmp/expected.npy")
    d = got - exp
    print("rel fro:", np.linalg.norm(d) / np.linalg.norm(exp))
    print("absmax rel:", np.abs(d).max() / np.abs(exp).max())



# revision 4
# speedup vs baseline: 1.3414x; 1.3414x over previous
"""Trainium2 Bass kernel v3 for nn_BoxEncoder (B=128, T=200, NC=3, NB=2, D=512, DH=256).

Data-parallel over batch: 16 batch items per core x 8 cores; partition
p = bt*8 + q.  Per partition: 75 dist tokens (+1 pad) and 150 box tokens
(+2 pad), processed as 4-slot transpose chunks.

v3 vs v2 (~2x): the whole kernel is built to keep the PE p-state warm
(cost model: matmuls run 2x faster once the PE has been ~continuously
busy for 3us; any long stall drops it back):
 - z is computed TRANSPOSED (weights-stationary): zT = W1bandedT @ cta_s,
   so gelu(zT) directly yields hT = the lhsT of the W2 matmuls.  The
   dma_start_transpose of h (SP-queue serial 1.24us each + 900ns sem) is
   gone - that chain caused the recurring 3-5us PE stalls in v2.
 - LN rstd is folded into the geometry features BEFORE the z matmul
   (f0..f9 *= rstd per token), so gelu needs no per-partition scale and
   batches [128,512] over 4 slots per call.  cx,cy are duplicated into
   f18,f19 (unscaled) for the center_w rows of the extras matmul.
 - all matmuls are full-K (no tile_position): banded *weights* (zero rows
   outside the slot's 32-band) instead of banded matmuls; the extras and
   z matmuls share one LDWEIGHTS of the feature chunk.
 - variance via a 12-slot x 10-feature gram pack (13 transposes instead
   of 38) and a single batched sqrt+reciprocal.
 - PE pre-warm: a dozen junk matmuls issued at t~1us keep the PE busy
   during the DVE feature-prep phase so the clock is warm when real
   matmuls start.
 - staging copies round-robin DVE/ACT/GPSIMD; out DMAs on the otherwise
   idle SP queue; one ACT table preload for Gelu right after the rstd
   sqrt so no table load lands inside the steady-state loop.
"""

import numpy as np
import ml_dtypes

B, T, NCAM, NB, D, DH = 128, 200, 3, 2, 512, 256
IW, IH = 640.0, 400.0
NCORES = 8
BPC = B // NCORES
JB = 150                  # real box slots per partition
JBP = 156                 # padded (38 chunks use 152; gram packs use 156)
JD = 75                   # real dist slots
JDP = 76                  # padded (19 chunks)
NCH = 38                  # box chunks
NDC = 19                  # dist chunks
NOCT = 19                 # box octs (2 chunks = 8 slots each; last has 6)
NGP = 13                  # gram packs (12 slots x 10 feats)

_CACHE = {}


def _build_nc():
    from contextlib import ExitStack
    import concourse.bacc as bacc
    import concourse.mybir as mybir
    import concourse.tile as tile

    f32 = mybir.dt.float32
    bf16 = mybir.dt.bfloat16
    A = mybir.AluOpType
    AF = mybir.ActivationFunctionType

    # bpk bf16 column offsets
    C_W1B = 0                       # 8 x 128 (band b: hi, lo)
    C_W2HI = C_W1B + 8 * 128
    C_W2LO = C_W2HI + 512
    C_W2XB = C_W2LO + 512           # 12 x 512 (cam c, band b)
    C_G = C_W2XB + 12 * 512
    NBF = C_G + 128

    nc = bacc.Bacc("TRN2", target_bir_lowering=False, debug=False,
                   num_devices=NCORES)
    fpk = nc.declare_dram_parameter("fpk", [128, 900 + 128], f32, isOutput=False)
    bpk = nc.declare_dram_parameter("bpk", [128, NBF], bf16, isOutput=False)
    out_d = nc.declare_dram_parameter("out", [BPC, 1800, D], bf16, isOutput=True)

    with ExitStack() as ctx:
        tc = ctx.enter_context(tile.TileContext(nc))
        cp = ctx.enter_context(tc.tile_pool(name="const", bufs=1))
        sc = ctx.enter_context(tc.tile_pool(name="scratch", bufs=1))
        # PSUM pools (8 banks): zp 4x[128,512]=4 + opa 3x[128,512]=3 +
        # tp 2x[128,128]=0.5
        zp = ctx.enter_context(tc.tile_pool(name="zp", bufs=4, space="PSUM"))
        opa = ctx.enter_context(tc.tile_pool(name="opa", bufs=3, space="PSUM"))
        tp = ctx.enter_context(tc.tile_pool(name="tp", bufs=1, space="PSUM"))
        gpck = ctx.enter_context(tc.tile_pool(name="gpck", bufs=2))
        octp = ctx.enter_context(tc.tile_pool(name="octp", bufs=4))
        cdp = ctx.enter_context(tc.tile_pool(name="cdp", bufs=1))
        htp = ctx.enter_context(tc.tile_pool(name="htp", bufs=8))
        bstg = ctx.enter_context(tc.tile_pool(name="bstage", bufs=3))
        dstg = ctx.enter_context(tc.tile_pool(name="dstage", bufs=3))

        fpack = cp.tile([128, 900 + 128], f32)
        nc.sync.dma_start(fpack[:], fpk[:])
        bpack = cp.tile([128, NBF], bf16)
        nc.sync.dma_start(bpack[:], bpk[:])

        raw = fpack[:, 0:900]
        idf = fpack[:, 900:1028]
        w1b = [(bpack[:, C_W1B + (2 * b) * 128: C_W1B + (2 * b + 1) * 128],
                bpack[:, C_W1B + (2 * b + 1) * 128: C_W1B + (2 * b + 2) * 128])
               for b in range(4)]
        w2hi = bpack[:, C_W2HI:C_W2HI + 512]
        w2lo = bpack[:, C_W2LO:C_W2LO + 512]
        w2xb = [[bpack[:, C_W2XB + (c * 4 + b) * 512: C_W2XB + (c * 4 + b + 1) * 512]
                 for b in range(4)] for c in range(3)]
        Gblk = bpack[:, C_G:C_G + 128]

        # ---------------- PE pre-warm: junk matmuls over bpack ----------
        for _ in range(13):
            wps = opa.tile([128, D], f32, tag="oa", name="oa")
            nc.tensor.matmul(wps[:], bpack[:, 0:128], bpack[:, 0:512],
                             start=True, stop=True)

        # ---------------- P1: feature planes ----------------
        TFB = cp.tile([128, JBP * 32], f32)
        TFD = cp.tile([128, JDP * 32], f32)
        TFb = TFB.rearrange("p (j f) -> p j f", f=32)
        TFd = TFD.rearrange("p (j f) -> p j f", f=32)
        # zeros: box f16,17 + f20..31 + pad slots; dist all but f16,f17
        nc.gpsimd.memset(TFb[:, :, 16:18], 0.0)
        nc.gpsimd.memset(TFb[:, :, 20:32], 0.0)
        nc.gpsimd.memset(TFb[:, JB:JBP, 0:16], 0.0)
        nc.gpsimd.memset(TFb[:, JB:JBP, 18:20], 0.0)
        nc.gpsimd.memset(TFd[:, :, 0:16], 0.0)
        nc.gpsimd.memset(TFd[:, :, 18:32], 0.0)
        nc.gpsimd.memset(TFd[:, :, 17], 1.0)

        TFr = TFb[:, 0:JB, :]
        raw6 = raw.rearrange("p (b s) -> p b s", s=6)
        rawp = raw.rearrange("p (m g s) -> p m g s", g=2, s=6)
        TFbp = TFB[:, 0:JB * 32].rearrange("p (m g f) -> p m g f", g=2, f=32)

        sPres = sc.tile([128, JB], f32)
        sKey = sc.tile([128, JB], f32)
        sSwap = sc.tile([128, JD], f32)
        sD = sc.tile([128, JD], f32)
        sSD = sc.tile([128, JD], f32)
        sw = [sc.tile([128, JB], f32, tag=f"swp{i}", name=f"swp{i}")
              for i in range(6)]
        sT0 = sc.tile([128, JB], f32)
        sT1 = sc.tile([128, JB], f32)

        nc.vector.tensor_tensor(sT0[:], raw6[:, :, 0], raw6[:, :, 1], A.add)
        nc.vector.tensor_tensor(sT1[:], raw6[:, :, 2], raw6[:, :, 3], A.add)
        nc.vector.tensor_tensor(sT0[:], sT0[:], sT1[:], A.add)
        nc.vector.tensor_scalar(sPres[:], sT0[:], 0.0, None, A.not_equal)
        nc.vector.scalar_tensor_tensor(sKey[:], sPres[:], -1000.0,
                                       raw6[:, :, 4], A.mult, A.add)
        sKeyp = sKey.rearrange("p (m g) -> p m g", g=2)
        nc.vector.tensor_tensor(sSwap[:], sKeyp[:, :, 1], sKeyp[:, :, 0], A.is_lt)

        for i in range(6):
            ve, vo = rawp[:, :, 0, i], rawp[:, :, 1, i]
            dst = sw[i].rearrange("p (m g) -> p m g", g=2)
            nc.vector.tensor_tensor(sD[:], vo, ve, A.subtract)
            nc.vector.tensor_tensor(sSD[:], sD[:], sSwap[:], A.mult)
            nc.vector.tensor_tensor(dst[:, :, 0], ve, sSD[:], A.add)
            nc.vector.tensor_tensor(dst[:, :, 1], vo, sSD[:], A.subtract)
        sPresP = sPres.rearrange("p (m g) -> p m g", g=2)
        nc.vector.tensor_tensor(sD[:], sPresP[:, :, 1], sPresP[:, :, 0], A.subtract)
        nc.vector.tensor_tensor(sSD[:], sD[:], sSwap[:], A.mult)
        nc.vector.tensor_tensor(TFbp[:, :, 0, 14], sPresP[:, :, 0], sSD[:], A.add)
        nc.vector.tensor_tensor(TFbp[:, :, 1, 14], sPresP[:, :, 1], sSD[:], A.subtract)

        sX1, sY1, sX2, sY2, sCat, sConf = sw
        nc.vector.tensor_scalar(TFr[:, :, 0], sX1[:], 1.0 / IW, None, A.mult)
        nc.vector.tensor_scalar(TFr[:, :, 1], sY1[:], 1.0 / IH, None, A.mult)
        nc.vector.tensor_scalar(TFr[:, :, 2], sX2[:], 1.0 / IW, None, A.mult)
        nc.vector.tensor_scalar(TFr[:, :, 3], sY2[:], 1.0 / IH, None, A.mult)
        nc.vector.tensor_tensor(TFr[:, :, 4], TFr[:, :, 2], TFr[:, :, 0], A.subtract)
        nc.vector.tensor_tensor(TFr[:, :, 5], TFr[:, :, 3], TFr[:, :, 1], A.subtract)
        nc.vector.tensor_tensor(TFr[:, :, 6], TFr[:, :, 0], TFr[:, :, 2], A.add)
        nc.vector.tensor_tensor(TFr[:, :, 7], TFr[:, :, 1], TFr[:, :, 3], A.add)
        nc.vector.tensor_tensor(TFr[:, :, 8], TFr[:, :, 4], TFr[:, :, 5], A.mult)
        sHp = sT0
        nc.vector.tensor_scalar(sHp[:], TFr[:, :, 5], 1e-6, None, A.add)
        sR = sT1
        nc.vector.reciprocal(sR[:], sHp[:])
        nc.vector.tensor_tensor(TFr[:, :, 9], TFr[:, :, 4], sR[:], A.mult)
        for k in range(3):
            nc.vector.scalar_tensor_tensor(TFr[:, :, 10 + k], sCat[:], float(k),
                                           TFr[:, :, 14], A.is_equal, A.mult)
        nc.vector.tensor_tensor(TFr[:, :, 13], sConf[:], TFr[:, :, 14], A.mult)
        nc.vector.tensor_scalar(TFr[:, :, 15], TFr[:, :, 14], -1.0, 1.0,
                                A.mult, A.add)
        # unscaled center copies for the extras matmul (rows 18,19)
        nc.scalar.copy(TFr[:, :, 18], TFr[:, :, 6])
        nc.scalar.copy(TFr[:, :, 19], TFr[:, :, 7])
        # dist features: f16 = 0.5*sqrt(dx2^2+dy2^2), f17 = 1 (memset above)
        sDx = sc.tile([128, JD], f32)
        sDy = sc.tile([128, JD], f32)
        nc.vector.tensor_tensor(sDx[:], TFbp[:, :, 0, 6], TFbp[:, :, 1, 6], A.subtract)
        nc.vector.tensor_tensor(sDy[:], TFbp[:, :, 0, 7], TFbp[:, :, 1, 7], A.subtract)
        nc.vector.tensor_tensor(sDx[:], sDx[:], sDx[:], A.mult)
        nc.vector.tensor_tensor(sDy[:], sDy[:], sDy[:], A.mult)
        nc.vector.tensor_tensor(sDx[:], sDx[:], sDy[:], A.add)
        nc.scalar.activation(TFd[:, 0:JD, 16], sDx[:], AF.Sqrt, scale=0.25)

        # ---------------- P2: gram variance ----------------
        v = sc.tile([128, 160], f32)
        copy_rr = [nc.vector.tensor_copy, nc.scalar.copy]

        # prepack geom features f0..9 of all 156 slots contiguously; each
        # 128-col transpose window overlaps 8 cols into the next pack, which
        # land on zero rows of Gblk (harmless).
        gprep = sc.tile([128, 13 * 120 + 8], f32)
        nc.vector.memset(gprep[:, 13 * 120:], 0.0)
        nc.gpsimd.tensor_copy(
            gprep[:, 0:1560].rearrange("p (j f) -> p j f", f=10),
            TFb[:, 0:156, 0:10])

        for gi in range(NGP):
            s0 = 12 * gi
            src = TFb[:, s0:s0 + 12, 0:10]
            pst = zp.tile([128, 512], f32, tag="z", name="z")
            ps = pst[:, 0:128]
            nc.tensor.transpose(ps[:], gprep[:, 120 * gi:120 * gi + 128], idf)
            pk = gpck.tile([128, 128], bf16, tag="gp", name="gp")
            copy_rr[gi % 2](pk[:], ps[:])
            yt = zp.tile([128, 512], f32, tag="z", name="z")
            y = yt[:, 0:128]
            nc.tensor.matmul(y, pk[:], Gblk, start=True, stop=True)
            tmp = sc.tile([128, 120], f32, tag="gtmp", name="gtmp")
            nc.vector.tensor_tensor(tmp[:], src, y[:, 0:120], A.mult)
            nc.vector.tensor_reduce(v[:, s0:s0 + 12],
                                    tmp.rearrange("p (j f) -> p j f", f=10),
                                    mybir.AxisListType.X, A.add)

        # ---------------- P2b: dist transposes ----------------
        cta_d = cp.tile([128, NDC * 128], bf16)
        for dc in range(NDC):
            ps = tp.tile([128, 128], f32, tag="tp", name="tp")
            nc.tensor.transpose(ps[:], TFd[:, 4 * dc:4 * dc + 4, :], idf)
            copy_rr[dc % 2](cta_d[:, dc * 128:(dc + 1) * 128], ps[:])

        # ---------------- P3: rstd + feature scale + gelu preload --------
        eps = sc.tile([128, 1], f32)
        nc.vector.memset(eps[:], 1e-5)
        sd = sc.tile([128, 156], f32)
        rstd = sc.tile([128, 156], f32)
        nc.scalar.activation(sd[:], v[:, 0:156], AF.Sqrt,
                             bias=eps[:], scale=1.0 / DH)
        nc.vector.reciprocal(rstd[:], sd[:])
        # preload the Gelu ACT table off the critical path
        gjunk = sc.tile([128, 8], bf16)
        nc.scalar.activation(gjunk[:], sd[:, 0:8], AF.Gelu)
        # scale geometry features f0..9 by rstd (per token)
        for f in range(10):
            nc.gpsimd.tensor_tensor(TFb[:, :, f], TFb[:, :, f], rstd[:], A.mult)

        # ---------------- P2c: dist W2 + staging ----------------
        dist_stage = {"tile": None, "fill": 0, "base": 0}
        vd = out_d[:, 0:600, :].rearrange("b (q r) d -> b q r d", q=8)

        def stage_dist(kd, o):
            if dist_stage["tile"] is None:
                dist_stage["tile"] = dstg.tile([128, 5 * D], bf16, tag="dstage",
                                               name="dstage")
                dist_stage["fill"] = 0
                dist_stage["base"] = kd
            fill = dist_stage["fill"]
            copy_rr[kd % 2](dist_stage["tile"][:, fill * D:(fill + 1) * D], o[:])
            dist_stage["fill"] = fill + 1
            if dist_stage["fill"] == 5:
                b0 = dist_stage["base"]
                nc.sync.dma_start(vd[:, :, b0:b0 + 5, :], dist_stage["tile"][:])
                dist_stage["tile"] = None

        for dc in range(NDC):
            for b in range(4):
                kd = 4 * dc + b
                if kd >= JD:
                    continue
                o = opa.tile([128, D], f32, tag="oa", name="oa")
                nc.tensor.matmul(o[:], cta_d[:, dc * 128:(dc + 1) * 128],
                                 w2xb[0][b], start=True, stop=True)
                stage_dist(kd, o)

        # ---------------- P4/P5: box pipeline ----------------
        vb = out_d[:, 600:1800, :].rearrange("b (q r) d -> b q r d", q=8)
        oct_tiles = {}     # o -> sbuf [128, 256] bf16 (chunks 2o | 2o+1)
        ht_tiles = {}      # o -> list of 4 sbuf [128, 512] bf16 (per band)
        box_stage = {"tile": None, "fill": 0, "base": 0}
        ccnt = {"i": 0}

        def emit_pass2_half(o, ci):
            if ci == 0:
                oct_tiles[o] = octp.tile([128, 256], bf16, tag="oct", name="oct")
            t = oct_tiles[o]
            c = 2 * o + ci
            ps = tp.tile([128, 128], f32, tag="tp", name="tp")
            nc.tensor.transpose(ps[:], TFb[:, 4 * c:4 * c + 4, :], idf)
            copy_rr[c % 2](t[:, ci * 128:(ci + 1) * 128], ps[:])

        def emit_z(o):
            rhs = oct_tiles[o]
            hts = []
            for b in range(4):
                zb = zp.tile([128, 512], f32, tag="z", name="z")
                nc.tensor.matmul(zb[:, 0:256], w1b[b][0], rhs[:],
                                 start=True, stop=True)
                nc.tensor.matmul(zb[:, 256:512], w1b[b][1], rhs[:],
                                 start=True, stop=True)
                ht = htp.tile([128, 512], bf16, tag="ht", name="ht")
                nc.scalar.activation(ht[:], zb[:], AF.Gelu)
                hts.append(ht)
            ht_tiles[o] = hts

        def flush_box(last_k):
            b0 = box_stage["base"]
            gsz = box_stage["fill"]
            nc.sync.dma_start(vb[:, :, b0:b0 + gsz, :],
                              box_stage["tile"][:, 0:gsz * D])
            box_stage["tile"] = None

        def emit_w2(o):
            hts = ht_tiles.pop(o)
            cchunk = oct_tiles[o]
            for ci in range(2):
                c = 2 * o + ci
                for b in range(4):
                    s = 4 * c + b
                    if s >= JB:
                        continue
                    ht = hts[b]
                    cam = (s % 6) // 2
                    ot = opa.tile([128, D], f32, tag="oa", name="oa")
                    nc.tensor.matmul(ot[:], ht[:, ci * 128:ci * 128 + 128],
                                     w2hi, start=True, stop=False)
                    nc.tensor.matmul(ot[:], ht[:, 256 + ci * 128:256 + ci * 128 + 128],
                                     w2lo, start=False, stop=False)
                    nc.tensor.matmul(ot[:], cchunk[:, ci * 128:(ci + 1) * 128],
                                     w2xb[cam][b], start=False, stop=True)
                    if box_stage["tile"] is None:
                        box_stage["tile"] = bstg.tile([128, 8 * D], bf16,
                                                      tag="bstage", name="bstage")
                        box_stage["fill"] = 0
                        box_stage["base"] = s
                    fill = box_stage["fill"]
                    copy_rr[ccnt["i"] % 2](
                        box_stage["tile"][:, fill * D:(fill + 1) * D], ot[:])
                    ccnt["i"] += 1
                    box_stage["fill"] = fill + 1
                    if box_stage["fill"] == 8 or s == JB - 1:
                        flush_box(s)
            oct_tiles.pop(o)

        for step in range(NOCT + 2):
            if step < NOCT:
                emit_pass2_half(step, 0)
            if 1 <= step <= NOCT:
                emit_z(step - 1)
            if step < NOCT:
                emit_pass2_half(step, 1)
            if step >= 2:
                emit_w2(step - 2)

    nc.compile()
    return nc


def _prep_inputs(inputs):
    f32 = np.float32
    bf = ml_dtypes.bfloat16
    scale = float(np.asarray(inputs["scale"]))

    W1p = np.zeros((32, DH), f32)
    W1p[0:10] = np.asarray(inputs["geom_w1"], f32)
    W1p[6] *= 0.5
    W1p[7] *= 0.5
    W1p -= W1p.mean(axis=1, keepdims=True)

    w1b_cols = []
    for b in range(4):
        hi = np.zeros((128, 128), f32)
        hi[32 * b:32 * b + 32] = W1p[:, :128]
        lo = np.zeros((128, 128), f32)
        lo[32 * b:32 * b + 32] = W1p[:, 128:]
        w1b_cols += [hi, lo]

    W2s = scale * np.asarray(inputs["geom_w2"], f32)
    w2hi, w2lo = W2s[:128], W2s[128:]

    cat_t = np.asarray(inputs["cat_table"], f32)
    cam_t = np.asarray(inputs["cam_table"], f32)
    bias_row = (np.asarray(inputs["geom_b2"], f32)
                + np.asarray(inputs["conf_b"], f32)
                + np.asarray(inputs["center_b"], f32))
    w2xb_cols = []
    for c in range(3):
        W2X = np.zeros((32, D), f32)
        W2X[10:13] = scale * cat_t
        W2X[13] = scale * np.asarray(inputs["conf_w"], f32)[0]
        W2X[14] = scale * (bias_row + cam_t[c])
        W2X[15] = np.asarray(inputs["missing_emb"], f32)[0]
        W2X[16] = np.asarray(inputs["dist_w"], f32)[0]
        W2X[17] = np.asarray(inputs["dist_b"], f32)
        W2X[18] = scale * np.asarray(inputs["center_w"], f32)[0] * 0.5
        W2X[19] = scale * np.asarray(inputs["center_w"], f32)[1] * 0.5
        for b in range(4):
            t = np.zeros((128, D), f32)
            t[32 * b:32 * b + 32] = W2X
            w2xb_cols.append(t)

    G10 = (W1p[0:10] @ W1p[0:10].T).astype(f32)
    Gblk = np.zeros((128, 128), f32)
    for s in range(12):
        Gblk[10 * s:10 * s + 10, 10 * s:10 * s + 10] = G10

    bpk = np.concatenate(w1b_cols + [w2hi, w2lo] + w2xb_cols + [Gblk],
                         axis=1).astype(bf)

    idf32 = np.eye(128, dtype=f32)
    box = np.asarray(inputs["box_data"], f32)
    fpks = []
    for c in range(NCORES):
        rawc = box[c * BPC:(c + 1) * BPC].reshape(BPC, T * 6, 6)
        rawc = rawc.reshape(BPC, 8, JB, 6).reshape(128, 900)
        fpks.append(np.ascontiguousarray(
            np.concatenate([rawc, idf32], axis=1), dtype=f32))
    return fpks, bpk


def _fast_path_ok(inputs):
    try:
        shapes = {
            "box_data": (B, T, 6, 6), "cat_table": (3, D), "geom_w1": (10, DH),
            "geom_b1": (DH,), "ln_g": (DH,), "ln_b": (DH,), "geom_w2": (DH, D),
            "geom_b2": (D,), "conf_w": (1, D), "conf_b": (D,),
            "center_w": (2, D), "center_b": (D,), "missing_emb": (1, D),
            "dist_w": (1, D), "dist_b": (D,), "cam_table": (NCAM, D),
        }
        for k, s in shapes.items():
            if tuple(np.asarray(inputs[k]).shape) != s:
                return False
        if not np.all(np.asarray(inputs["geom_b1"]) == 0):
            return False
        if not np.all(np.asarray(inputs["ln_g"]) == 1):
            return False
        if not np.all(np.asarray(inputs["ln_b"]) == 0):
            return False
        return True
    except Exception:
        return False


def _numpy_fallback(inputs):
    import math
    f32 = np.float32
    inp = {k: np.asarray(v) for k, v in inputs.items()}
    coords = inp["box_data"][..., :4].astype(f32)
    category = inp["box_data"][..., 4].astype(np.int32)
    conf = inp["box_data"][..., 5].astype(f32)
    norm = np.array([IW, IH, IW, IH], f32)
    cn = (coords / norm).reshape(B, T, NCAM, NB, 4)
    category = category.reshape(B, T, NCAM, NB)
    conf = conf.reshape(B, T, NCAM, NB, 1)
    presence = (cn.sum(-1) != 0).astype(f32)
    sort_key = category.astype(f32) + (1.0 - presence) * 1000.0
    idx = np.argsort(sort_key, axis=-1, kind="stable")
    cn = np.take_along_axis(cn, idx[..., None], axis=-2)
    category = np.take_along_axis(category, idx, axis=-1)
    conf = np.take_along_axis(conf, idx[..., None], axis=-2)
    presence = (cn.sum(-1) != 0).astype(f32)[..., None]
    x1, y1, x2, y2 = cn[..., 0], cn[..., 1], cn[..., 2], cn[..., 3]
    w, h = x2 - x1, y2 - y1
    cx, cy = (x1 + x2) * 0.5, (y1 + y2) * 0.5
    area, aspect = w * h, w / (h + 1e-6)
    dx, dy = cx[..., 0] - cx[..., 1], cy[..., 0] - cy[..., 1]
    dist = np.sqrt(dx * dx + dy * dy)[..., None]
    dist_tok = dist @ inp["dist_w"].astype(f32) + inp["dist_b"].astype(f32)
    geom = np.stack([x1, y1, x2, y2, w, h, cx, cy, area, aspect], axis=-1)
    z = geom @ inp["geom_w1"].astype(f32) + inp["geom_b1"].astype(f32)
    mu = z.mean(-1, keepdims=True)
    var = ((z - mu) ** 2).mean(-1, keepdims=True)
    xh = (z - mu) / np.sqrt(var + 1e-5) * inp["ln_g"].astype(f32) + inp["ln_b"].astype(f32)
    try:
        from scipy.special import erf as _erf
        g = xh * 0.5 * (1.0 + _erf(xh / np.sqrt(2.0)))
    except Exception:
        verf = np.vectorize(math.erf)
        g = xh * 0.5 * (1.0 + verf(xh / np.sqrt(2.0)))
    geom_p = g @ inp["geom_w2"].astype(f32) + inp["geom_b2"].astype(f32)
    cat_emb = inp["cat_table"].astype(f32)[category]
    conf_p = conf @ inp["conf_w"].astype(f32) + inp["conf_b"].astype(f32)
    center_p = np.stack([cx, cy], axis=-1) @ inp["center_w"].astype(f32) + inp["center_b"].astype(f32)
    cam_emb = inp["cam_table"].astype(f32).reshape(1, 1, NCAM, 1, D)
    tok = (geom_p + cat_emb + conf_p + center_p + cam_emb) * float(inp["scale"])
    tok = np.where(presence == 0, inp["missing_emb"].astype(f32)[0], tok)
    out = np.concatenate([dist_tok.reshape(B, T * NCAM, D),
                          tok.reshape(B, T * NCAM * NB, D)], axis=1)
    return out.astype(np.float32)


def _run(inputs, trace=False, tmpdir=None):
    from concourse.bass_utils import run_bass_kernel_spmd

    if "nc" not in _CACHE:
        _CACHE["nc"] = _build_nc()
    nc = _CACHE["nc"]

    fpks, bpk = _prep_inputs(inputs)
    in_maps = [{"fpk": fpks[c], "bpk": bpk} for c in range(NCORES)]
    res = run_bass_kernel_spmd(nc, in_maps, list(range(NCORES)),
                               trace=trace, tmpdir=tmpdir)
    out = np.concatenate([np.asarray(res.results[c]["out"])
                          for c in range(NCORES)], axis=0)
    return out.astype(np.float32), res


def kernel(**inputs):
    if not _fast_path_ok(inputs):
        return _numpy_fallback(inputs)
    out, _ = _run(inputs)
    return out


if __name__ == "__main__":
    import reference as ref
    inputs = {k: np.asarray(v) for k, v in ref.setup_inputs().items()}
    got = kernel(**inputs)
    exp = np.load("/tmp/expected.npy")
    d = got - exp
    print("rel fro:", np.linalg.norm(d) / np.linalg.norm(exp))
    print("absmax rel:", np.abs(d).max() / np.abs(exp).max())
